# revision 10
# baseline (speedup 1.0000x reference)
"""AttentionMIL Trainium2 kernel.

Math per bag of 512 instances:
    emb    = relu(x @ w_enc + b_enc)            [512, 128]
    a      = tanh(emb @ w_att + b_att)          [512, 64]
    logits = a @ w_score (+ b_score dropped: softmax shift-invariant)
    e      = exp(logits - const)                (denom = sum e)
    score  = (sum_i e_i * (emb_i @ w_cls)) / denom + b_cls   [2]
  (w_cls folded through the attention sum: the bag embedding is never
   materialized; u = w_cls^T @ emb is a cheap PE matmul overlapped with
   tanh, and the weighted sum becomes a [2, 512] DVE dot with e.)

Distribution: data-parallel over bags; 8 cores x 8 bags, weights
replicated, no cross-core communication. Host pre-transposes each
core's x shard to x^T and casts to fp8e4 (TRN E4M3), quartering the
HBM traffic vs f32 (rel err ~6e-3 against the 2e-2 gate).

Design notes:
 - 4 slabs x 1 bag-pair: head/tail work for slab s becomes available
   while slab s+1 is still streaming, filling the encoder's DMA-wait
   gaps (keeps PE duty high so the HAM clock gate stays at 8/8).
 - x split across queues (sync HWDGE: slabs 0,3 + out; gpsimd SWDGE:
   slabs 1,2; scalar HWDGE: weights then activations only): each
   dma_start costs ~0.7us of issue time on its queue regardless of
   size, so one ring cannot feed 16 chunks, and the scalar queue must
   stay short or relu s0 would sit behind x-DMA issues.
 - Encoder: fp8 DoubleRow matmuls (K=256/pass; w_enc pre-packed
   [p, group, ko, m] on host), 2 MMs per 256KB chunk, psum pair tile
   double-buffered across slabs.
 - PE warmup (~28 dummy 128-col MMs) bridges the DMA ramp and flips
   the HAM clock gate 4/8 -> 8/8 (~3us of dense activity required).
 - Per-bag tail: ws2 stationary has w_score duplicated in BOTH columns
   for one bag's partition range -> psl [2, 512] with identical rows;
   exp of that gives e duplicated on 2 partitions AND (via accum_out)
   the denominator duplicated per class lane -- exactly the layout the
   score dot and the final normalization need. No partition broadcasts.
 - u matmuls are emitted between watt and ws2 so the PE does useful
   work while the scalar engine runs tanh.
 - Drain pair (slab 3) is column-halved so the serial tail chain
   pipelines across engines; its g3 chunk arrives as 4 x 64KB pieces
   in (h, m) order so relu h0 starts before the last bytes land.
"""

import sys

sys.path.insert(0, "/opt/trn_rl_repo")

import numpy as np

N_INST = 32768
N_BAGS = 64
D_IN = 1024
D_EMB = 128
D_ATT = 64
N_CLS = 2

N_CORES = 8
BAGS_PER_CORE = N_BAGS // N_CORES          # 8
INST_PER_BAG = N_INST // N_BAGS            # 512
INST_PER_CORE = N_INST // N_CORES          # 4096
N_GRP = 4                                  # DoubleRow k-groups (256 each)
N_SLABS = 4                                # 1 bag-pair per slab
BAGS_PER_SLAB = BAGS_PER_CORE // N_SLABS   # 2
SLAB_INST = BAGS_PER_SLAB * INST_PER_BAG   # 1024
N_PAIRS = N_SLABS                          # pair J == slab s
N_WARM = 28

_CACHE = {}


def _build():
    import concourse.bacc as bacc
    import concourse.mybir as mybir
    import concourse.tile as tile

    f32 = mybir.dt.float32
    f32r = mybir.dt.float32r
    bf16 = mybir.dt.bfloat16
    fp8 = mybir.dt.float8e4
    AF = mybir.ActivationFunctionType
    ALU = mybir.AluOpType
    DR = mybir.MatmulPerfMode.DoubleRow

    nc = bacc.Bacc("TRN2", target_bir_lowering=False, debug=False,
                   enable_asserts=False, num_devices=N_CORES)

    xt = nc.dram_tensor("xt", [D_IN, INST_PER_CORE], fp8, kind="ExternalInput")
    # w_enc pre-packed [128 p, 4 g, 2 ko, 128 m]: w_enc[(2g+ko)*128+p, m]
    w_enc = nc.dram_tensor("w_enc", [128, N_GRP, 2, D_EMB], fp8,
                           kind="ExternalInput")
    # bf16 consts [128, 70]: 0:64 w_att | 64:66 ws2 bag0 (w_score on
    # partitions 0:64, both cols) | 66:68 ws2 bag1 (partitions 64:128) |
    # 68:70 w_cls
    cb16 = nc.dram_tensor("cb16", [128, 70], bf16, kind="ExternalInput")
    # f32 consts [128, 12]: col0 b_enc, col1 b_att2, cols 2:10 b_cls
    # replicated per bag on partitions 0:2 (classes)
    cf32 = nc.dram_tensor("cf32", [128, 12], f32, kind="ExternalInput")
    out = nc.dram_tensor("out", [2, BAGS_PER_CORE], f32,
                         kind="ExternalOutput")

    xt_re = xt[:, :].rearrange("(c p) i -> p c i", p=128)
    HALF = INST_PER_BAG // 2

    with tile.TileContext(nc) as tc:
        with (
            tc.tile_pool(name="const", bufs=1) as const,
            tc.tile_pool(name="xs", bufs=4) as xs_pool,
            tc.tile_pool(name="work", bufs=2) as work,
            tc.tile_pool(name="ps", bufs=2, space="PSUM") as ps,
            tc.tile_pool(name="psa", bufs=1, space="PSUM") as psa,
            tc.tile_pool(name="psl", bufs=1, space="PSUM") as psl,
            tc.tile_pool(name="psu", bufs=2, space="PSUM") as psu,
        ):
            # ---- PE warmup: flip HAM to 8/8 during the DMA wait ----
            warm_w = const.tile([128, 128], bf16)
            nc.vector.memset(warm_w, 0.0)
            warm_w2 = const.tile([128, 128], bf16)
            nc.vector.memset(warm_w2, 0.0)
            ps_warm = psa.tile([128, 512], f32, tag="a", name="ps_warm")
            for i in range(N_WARM):
                c0 = (i % 4) * 128
                nc.tensor.matmul(ps_warm[:, c0:c0 + 128],
                                 warm_w if i % 2 == 0 else warm_w2,
                                 warm_w2 if i % 2 == 0 else warm_w,
                                 start=True, stop=True, skip_group_check=True)

            # ---- x chunks: sync ring = slabs 0,2; scalar = 1,3 ----
            xg = [[None] * N_GRP for _ in range(N_SLABS)]

            def emit_x_slab(s, eng):
                i0 = s * SLAB_INST
                for g in range(N_GRP):
                    t = xs_pool.tile([128, 2, SLAB_INST], fp8, tag=f"x{g}",
                                     name=f"xs{s}g{g}")
                    if s == 0 and g == 0:
                        # per-bag halves: the first encoder matmul only
                        # needs bag 0
                        for m in range(2):
                            sl = slice(m * INST_PER_BAG, (m + 1) * INST_PER_BAG)
                            eng.dma_start(
                                out=t[:, :, sl],
                                in_=xt_re[:, 2 * g:2 * g + 2,
                                          i0 + m * INST_PER_BAG:
                                          i0 + (m + 1) * INST_PER_BAG])
                    elif s == N_SLABS - 1 and g == N_GRP - 1:
                        # drain chunk: 4 x 64KB pieces in (h, m) order so
                        # the halved tail's relu h0 starts early
                        for h in range(2):
                            for m in range(2):
                                c0 = m * INST_PER_BAG + h * HALF
                                eng.dma_start(
                                    out=t[:, :, c0:c0 + HALF],
                                    in_=xt_re[:, 2 * g:2 * g + 2,
                                              i0 + c0:i0 + c0 + HALF])
                    else:
                        eng.dma_start(
                            out=t,
                            in_=xt_re[:, 2 * g:2 * g + 2, i0:i0 + SLAB_INST])
                    xg[s][g] = t

            emit_x_slab(0, nc.sync)
            # ---- replicated weights on the scalar HWDGE ring (scalar
            # then runs only activations, so its queue stays short) ----
            wenc_sb = const.tile([128, N_GRP, 2, D_EMB], fp8)
            nc.scalar.dma_start(out=wenc_sb, in_=w_enc[:, :, :, :])
            cb16_sb = const.tile([128, 70], bf16)
            nc.scalar.dma_start(out=cb16_sb, in_=cb16[:, :])
            cf32_sb = const.tile([128, 12], f32)
            nc.scalar.dma_start(out=cf32_sb, in_=cf32[:, :])
            emit_x_slab(1, nc.gpsimd)
            emit_x_slab(2, nc.gpsimd)
            emit_x_slab(3, nc.sync)

            watt_ap = cb16_sb[:, 0:64]
            ws2_ap = [cb16_sb[:, 64:66], cb16_sb[:, 66:68]]
            wcls_ap = cb16_sb[:, 68:70]
            benc_ap = cf32_sb[:, 0:1]
            batt2_ap = cf32_sb[:, 1:2]
            bcls_ap = cf32_sb[0:2, 2:10]

            den_all = const.tile([2, BAGS_PER_CORE], f32)
            sc_all = const.tile([2, BAGS_PER_CORE], f32)
            rden = const.tile([2, BAGS_PER_CORE], f32r)

            def emit_enc(s):
                pair = ps.tile([128, 2, INST_PER_BAG], f32, tag="e",
                               name=f"pse{s}")
                for g in range(N_GRP):
                    if s == N_SLABS - 1 and g == N_GRP - 1:
                        for h in range(2):
                            for m in range(2):
                                c0 = h * HALF
                                nc.tensor.matmul(
                                    pair[:, m, c0:c0 + HALF],
                                    wenc_sb[:, g, :, :],
                                    xg[s][g][:, :,
                                             m * INST_PER_BAG + c0:
                                             m * INST_PER_BAG + c0 + HALF],
                                    start=False, stop=True,
                                    perf_mode=DR, skip_group_check=True)
                    else:
                        for m in range(2):
                            nc.tensor.matmul(
                                pair[:, m, :],
                                wenc_sb[:, g, :, :],
                                xg[s][g][:, :, m * INST_PER_BAG:
                                         (m + 1) * INST_PER_BAG],
                                start=(g == 0), stop=(g == N_GRP - 1),
                                perf_mode=DR,
                                skip_group_check=(s == N_SLABS - 1))
                return pair

            def emit_head(s, pair, halve=False):
                """relu -> watt -> tanh (per column range)."""
                embT2 = work.tile([128, 2, INST_PER_BAG], bf16, tag="emb",
                                  name=f"emb{s}")
                ps_a = psa.tile([D_EMB, INST_PER_BAG], f32, tag="a",
                                name=f"psa{s}")
                aT2 = work.tile([128, INST_PER_BAG], bf16, tag="aT",
                                name=f"aT{s}")
                ranges = [slice(h * HALF, (h + 1) * HALF)
                          for h in range(2)] if halve \
                    else [slice(0, INST_PER_BAG)]
                for sl in ranges:
                    nc.scalar.activation(embT2[:, :, sl], pair[:, :, sl],
                                         AF.Relu, bias=benc_ap, scale=1.0)
                    nc.tensor.matmul(ps_a[0:64, sl], watt_ap,
                                     embT2[:, 0, sl], start=True, stop=True,
                                     tile_position=(0, 0))
                    nc.tensor.matmul(ps_a[64:128, sl], watt_ap,
                                     embT2[:, 1, sl], start=True, stop=True,
                                     tile_position=(0, 64))
                    nc.scalar.activation(aT2[:, sl], ps_a[:, sl], AF.Tanh,
                                         bias=batt2_ap, scale=1.0)
                return embT2, aT2

            def emit_bags(s, embT2, aT2, halve=False):
                """u -> ws2 -> exp -> score dot, per bag."""
                if halve:
                    denp = work.tile([2, 2, 2], f32, tag="denp",
                                     name=f"denp{s}")
                    scp = work.tile([2, 2, 2], f32, tag="scp",
                                    name=f"scp{s}")
                ranges = [slice(h * HALF, (h + 1) * HALF)
                          for h in range(2)] if halve \
                    else [slice(0, INST_PER_BAG)]
                us = {m: psu.tile([2, INST_PER_BAG], f32, tag="u",
                                  name=f"u{s}{m}") for m in range(2)}
                for hi, sl in enumerate(ranges):
                    for m in range(2):
                        nc.tensor.matmul(us[m][:, sl], wcls_ap,
                                         embT2[:, m, sl], start=True,
                                         stop=True, skip_group_check=True)
                    for m in range(2):
                        j = 2 * s + m
                        ps_l = psl.tile([2, INST_PER_BAG], f32, tag="l",
                                        name=f"psl{s}{m}")
                        e2 = work.tile([2, INST_PER_BAG], bf16,
                                       tag=f"e2{m}", name=f"e2{s}{m}")
                        nc.tensor.matmul(ps_l[:, sl], ws2_ap[m], aT2[:, sl],
                                         start=True, stop=True,
                                         skip_group_check=True)
                        acc = denp[:, m, hi:hi + 1] if halve \
                            else den_all[0:2, j:j + 1]
                        nc.scalar.activation(e2[:, sl], ps_l[:, sl], AF.Exp,
                                             scale=1.0, accum_out=acc)
                        sacc = scp[:, m, hi:hi + 1] if halve \
                            else sc_all[0:2, j:j + 1]
                        scr = work.tile([2, INST_PER_BAG], bf16, tag="scr",
                                        name=f"scr{s}{m}{hi}")
                        nc.vector.scalar_tensor_tensor(
                            out=scr[:, sl], in0=us[m][:, sl], scalar=1.0,
                            in1=e2[:, sl], op0=ALU.mult, op1=ALU.mult,
                            accum_out=sacc)
                if halve:
                    for m in range(2):
                        j = 2 * s + m
                        nc.vector.tensor_add(den_all[0:2, j:j + 1],
                                             denp[:, m, 0:1], denp[:, m, 1:2])
                        nc.vector.tensor_add(sc_all[0:2, j:j + 1],
                                             scp[:, m, 0:1], scp[:, m, 1:2])

            # order: enc s | head s | enc s+1 | bags s | head s+1 | ...
            prev = None
            for s in range(N_SLABS):
                pair = emit_enc(s)
                if prev is not None:
                    emit_bags(s - 1, *prev)
                    if s == N_SLABS - 1:
                        # reciprocal of the first 6 denominators off the
                        # critical drain chain
                        with nc.allow_low_precision(
                                reason="1/denom at f32r, ~1e-4 rel"):
                            nc.vector.reciprocal(rden[:, 0:6],
                                                 den_all[:, 0:6])
                prev = emit_head(s, pair, halve=(s == N_SLABS - 1))
            emit_bags(N_SLABS - 1, *prev, halve=True)

            # ---- score normalization + output ----
            with nc.allow_low_precision(reason="1/denom at f32r, ~1e-4 rel"):
                nc.vector.reciprocal(rden[:, 6:8], den_all[:, 6:8])
            s_n = const.tile([2, BAGS_PER_CORE], f32)
            nc.vector.tensor_mul(s_n, sc_all, rden)
            scores = const.tile([2, BAGS_PER_CORE], f32)
            nc.vector.tensor_add(scores, s_n, bcls_ap)
            nc.sync.dma_start(out=out[:, :], in_=scores)

    nc.compile()
    return nc


def _prep_shared(w_enc, b_enc, w_att, b_att, w_score, w_cls, b_cls):
    import ml_dtypes

    wenc_dr = np.ascontiguousarray(
        w_enc.reshape(N_GRP, 2, 128, D_EMB).transpose(2, 0, 1, 3)
    ).astype(ml_dtypes.float8_e4m3)

    cb16 = np.zeros((128, 70), dtype=np.float32)
    cb16[:, 0:64] = w_att
    cb16[0:64, 64] = w_score
    cb16[0:64, 65] = w_score
    cb16[64:128, 66] = w_score
    cb16[64:128, 67] = w_score
    cb16[:, 68:70] = w_cls
    cb16 = cb16.astype(ml_dtypes.bfloat16)

    cf32 = np.zeros((128, 12), dtype=np.float32)
    cf32[:, 0] = b_enc
    cf32[0:64, 1] = b_att
    cf32[64:128, 1] = b_att
    cf32[0:2, 2:10] = np.repeat(b_cls[:, None], BAGS_PER_CORE, axis=1)
    return {"w_enc": wenc_dr, "cb16": cb16, "cf32": cf32}


def make_in_maps(inputs):
    import ml_dtypes

    x = np.asarray(inputs["x"], dtype=np.float32)
    shared = _prep_shared(
        np.asarray(inputs["w_enc"], dtype=np.float32),
        np.asarray(inputs["b_enc"], dtype=np.float32),
        np.asarray(inputs["w_att"], dtype=np.float32),
        np.asarray(inputs["b_att"], dtype=np.float32),
        np.asarray(inputs["w_score"], dtype=np.float32),
        np.asarray(inputs["w_cls"], dtype=np.float32),
        np.asarray(inputs["b_cls"], dtype=np.float32),
    )
    in_maps = []
    for c in range(N_CORES):
        xs = x[c * INST_PER_CORE:(c + 1) * INST_PER_CORE]
        xt = np.ascontiguousarray(xs.T)
        np.clip(xt, -240.0, 240.0, out=xt)
        in_maps.append({"xt": xt.astype(ml_dtypes.float8_e4m3), **shared})
    return in_maps


def unpack_out(res):
    outs = []
    for c in range(N_CORES):
        o = np.asarray(res.results[c]["out"], dtype=np.float32)
        outs.append(o.T)                   # [2, 8] -> [8, 2]
    return np.ascontiguousarray(np.concatenate(outs, axis=0))


def _numpy_fallback(x, seg, w_enc, b_enc, w_att, b_att, w_score, b_score,
                    w_cls, b_cls):
    emb = np.maximum(x @ w_enc + b_enc, 0.0)
    a = np.tanh(emb @ w_att + b_att)
    logits = a @ w_score + b_score[0]
    out = np.zeros((N_BAGS, N_CLS), dtype=np.float32)
    for bag in range(N_BAGS):
        mask = seg == bag
        lg = logits[mask]
        e = np.exp(lg - lg.max())
        attn = e / e.sum()
        bag_emb = attn @ emb[mask]
        out[bag] = bag_emb @ w_cls + b_cls
    return out


def kernel(**inputs):
    from concourse.bass_utils import run_bass_kernel_spmd

    seg = np.asarray(inputs["seg"], dtype=np.int32)
    expected_seg = np.repeat(np.arange(N_BAGS, dtype=np.int32), INST_PER_BAG)
    if not np.array_equal(seg, expected_seg):
        return _numpy_fallback(
            np.asarray(inputs["x"], dtype=np.float32), seg,
            *[np.asarray(inputs[k], dtype=np.float32) for k in
              ("w_enc", "b_enc", "w_att", "b_att", "w_score", "b_score",
               "w_cls", "b_cls")])

    if "nc" not in _CACHE:
        _CACHE["nc"] = _build()
    nc = _CACHE["nc"]
    in_maps = make_in_maps(inputs)
    res = run_bass_kernel_spmd(nc, in_maps, core_ids=list(range(N_CORES)))
    return unpack_out(res)


# revision 12
# speedup vs baseline: 1.0153x; 1.0153x over previous
"""AttentionMIL Trainium2 kernel.

Math per bag of 512 instances:
    emb    = relu(x @ w_enc + b_enc)            [512, 128]
    a      = tanh(emb @ w_att + b_att)          [512, 64]
    logits = a @ w_score (+ b_score dropped: softmax shift-invariant)
    e      = exp(logits - const)                (denom = sum e)
    score  = (sum_i e_i * (emb_i @ w_cls)) / denom + b_cls   [2]
  (w_cls folded through the attention sum: the bag embedding is never
   materialized; u = w_cls^T @ emb is a cheap PE matmul overlapped with
   tanh, and the weighted sum becomes a [2, 512] DVE dot with e.)

Distribution: data-parallel over bags; 8 cores x 8 bags, weights
replicated, no cross-core communication. Host pre-transposes each
core's x shard to x^T and casts to fp8e4 (TRN E4M3), quartering the
HBM traffic vs f32 (rel err ~6e-3 against the 2e-2 gate).

Design notes:
 - 4 slabs x 1 bag-pair: head/tail work for slab s becomes available
   while slab s+1 is still streaming, filling the encoder's DMA-wait
   gaps (keeps PE duty high so the HAM clock gate stays at 8/8).
 - x split across queues (sync HWDGE: slabs 0,3 + out; gpsimd SWDGE:
   slabs 1,2; scalar HWDGE: weights then activations only): each
   dma_start costs ~0.7us of issue time on its queue regardless of
   size, so one ring cannot feed 16 chunks, and the scalar queue must
   stay short or relu s0 would sit behind x-DMA issues.
 - Encoder: fp8 DoubleRow matmuls (K=256/pass; w_enc pre-packed
   [p, group, ko, m] on host), 2 MMs per 256KB chunk, psum pair tile
   double-buffered across slabs.
 - PE warmup (~28 dummy 128-col MMs) bridges the DMA ramp and flips
   the HAM clock gate 4/8 -> 8/8 (~3us of dense activity required).
 - Per-bag tail: ws2 stationary has w_score duplicated in BOTH columns
   for one bag's partition range -> psl [2, 512] with identical rows;
   exp of that gives e duplicated on 2 partitions AND (via accum_out)
   the denominator duplicated per class lane -- exactly the layout the
   score dot and the final normalization need. No partition broadcasts.
 - u matmuls are emitted between watt and ws2 so the PE does useful
   work while the scalar engine runs tanh.
 - Drain pair (slab 3) is column-halved so the serial tail chain
   pipelines across engines; its g3 chunk arrives as 4 x 64KB pieces
   in (h, m) order so relu h0 starts before the last bytes land.
"""

import sys

sys.path.insert(0, "/opt/trn_rl_repo")

import numpy as np

N_INST = 32768
N_BAGS = 64
D_IN = 1024
D_EMB = 128
D_ATT = 64
N_CLS = 2

N_CORES = 8
BAGS_PER_CORE = N_BAGS // N_CORES          # 8
INST_PER_BAG = N_INST // N_BAGS            # 512
INST_PER_CORE = N_INST // N_CORES          # 4096
N_GRP = 4                                  # DoubleRow k-groups (256 each)
N_SLABS = 4                                # 1 bag-pair per slab
BAGS_PER_SLAB = BAGS_PER_CORE // N_SLABS   # 2
SLAB_INST = BAGS_PER_SLAB * INST_PER_BAG   # 1024
N_PAIRS = N_SLABS                          # pair J == slab s
N_WARM = 38

_CACHE = {}


def _build():
    import concourse.bacc as bacc
    import concourse.mybir as mybir
    import concourse.tile as tile

    f32 = mybir.dt.float32
    f32r = mybir.dt.float32r
    bf16 = mybir.dt.bfloat16
    fp8 = mybir.dt.float8e4
    AF = mybir.ActivationFunctionType
    ALU = mybir.AluOpType
    DR = mybir.MatmulPerfMode.DoubleRow

    nc = bacc.Bacc("TRN2", target_bir_lowering=False, debug=False,
                   enable_asserts=False, num_devices=N_CORES)

    xt = nc.dram_tensor("xt", [D_IN, INST_PER_CORE], fp8, kind="ExternalInput")
    # w_enc pre-packed [128 p, 4 g, 2 ko, 128 m]: w_enc[(2g+ko)*128+p, m]
    w_enc = nc.dram_tensor("w_enc", [128, N_GRP, 2, D_EMB], fp8,
                           kind="ExternalInput")
    # bf16 consts [128, 70]: 0:64 w_att | 64:66 ws2 bag0 (w_score on
    # partitions 0:64, both cols) | 66:68 ws2 bag1 (partitions 64:128) |
    # 68:70 w_cls
    cb16 = nc.dram_tensor("cb16", [128, 70], bf16, kind="ExternalInput")
    # f32 consts [128, 12]: col0 b_enc, col1 b_att2, cols 2:10 b_cls
    # replicated per bag on partitions 0:2 (classes)
    cf32 = nc.dram_tensor("cf32", [128, 12], f32, kind="ExternalInput")
    out = nc.dram_tensor("out", [2, BAGS_PER_CORE], f32,
                         kind="ExternalOutput")

    xt_re = xt[:, :].rearrange("(c p) i -> p c i", p=128)
    HALF = INST_PER_BAG // 2

    with tile.TileContext(nc) as tc:
        with (
            tc.tile_pool(name="const", bufs=1) as const,
            tc.tile_pool(name="xs", bufs=4) as xs_pool,
            tc.tile_pool(name="work", bufs=2) as work,
            tc.tile_pool(name="ps", bufs=2, space="PSUM") as ps,
            tc.tile_pool(name="psa", bufs=1, space="PSUM") as psa,
            tc.tile_pool(name="psl", bufs=1, space="PSUM") as psl,
            tc.tile_pool(name="psu", bufs=2, space="PSUM") as psu,
        ):
            # ---- PE warmup: flip HAM to 8/8 during the DMA wait ----
            warm_w = const.tile([128, 128], bf16)
            nc.vector.memset(warm_w, 0.0)
            warm_w2 = const.tile([128, 128], bf16)
            nc.vector.memset(warm_w2, 0.0)
            ps_warm = psa.tile([128, 512], f32, tag="a", name="ps_warm")
            for i in range(N_WARM):
                c0 = (i % 4) * 128
                nc.tensor.matmul(ps_warm[:, c0:c0 + 128],
                                 warm_w if i % 2 == 0 else warm_w2,
                                 warm_w2 if i % 2 == 0 else warm_w,
                                 start=True, stop=True, skip_group_check=True)

            # ---- x chunks ----
            # Each dma_start costs ~0.65us of issue time on its HWDGE
            # ring regardless of size, so: slab 0 streams as 256KB
            # group-chunks (fine ramp granularity), slabs 1-3 as 512KB
            # two-group chunks. sync ring: s0, s1, s3, out (pure DMA
            # queue); scalar ring: weights + s2 upfront, then only
            # activations. SWDGE (gpsimd) is useless here: ~2.6us of Q7
            # descriptor generation per 256-descriptor chunk (measured).
            xg = [[None] * N_GRP for _ in range(N_SLABS)]

            def emit_x_s0(eng):
                for g in range(N_GRP):
                    t = xs_pool.tile([128, 2, SLAB_INST], fp8,
                                     tag=f"x0g{g}", name=f"xs0g{g}")
                    if g == 0:
                        # per-bag halves: the first encoder matmul only
                        # needs bag 0
                        for m in range(2):
                            sl = slice(m * INST_PER_BAG,
                                       (m + 1) * INST_PER_BAG)
                            eng.dma_start(out=t[:, :, sl],
                                          in_=xt_re[:, 0:2, sl])
                    else:
                        eng.dma_start(
                            out=t, in_=xt_re[:, 2 * g:2 * g + 2,
                                             0:SLAB_INST])
                    xg[0][g] = t

            def emit_x_mid(s, eng):
                i0 = s * SLAB_INST
                for half in range(2):
                    t = xs_pool.tile([128, 4, SLAB_INST], fp8,
                                     tag=f"x{s}c{half}", name=f"xs{s}c{half}")
                    eng.dma_start(
                        out=t, in_=xt_re[:, 4 * half:4 * half + 4,
                                         i0:i0 + SLAB_INST])
                    for gg in range(2):
                        xg[s][2 * half + gg] = t[:, 2 * gg:2 * gg + 2, :]

            def emit_x_s3(eng):
                s = N_SLABS - 1
                i0 = s * SLAB_INST
                t = xs_pool.tile([128, 4, SLAB_INST], fp8, tag="x3c0",
                                 name="xs3c0")
                eng.dma_start(out=t,
                              in_=xt_re[:, 0:4, i0:i0 + SLAB_INST])
                xg[s][0] = t[:, 0:2, :]
                xg[s][1] = t[:, 2:4, :]
                t2 = xs_pool.tile([128, 2, SLAB_INST], fp8, tag="x3g2",
                                  name="xs3g2")
                eng.dma_start(out=t2, in_=xt_re[:, 4:6, i0:i0 + SLAB_INST])
                xg[s][2] = t2
                # drain chunk: 4 x 64KB pieces in (h, m) order so the
                # halved tail's relu h0 starts early
                t3 = xs_pool.tile([128, 2, SLAB_INST], fp8, tag="x3g3",
                                  name="xs3g3")
                for h in range(2):
                    for m in range(2):
                        c0 = m * INST_PER_BAG + h * HALF
                        eng.dma_start(out=t3[:, :, c0:c0 + HALF],
                                      in_=xt_re[:, 6:8,
                                                i0 + c0:i0 + c0 + HALF])
                xg[s][3] = t3

            emit_x_s0(nc.sync)
            # ---- replicated weights on the scalar HWDGE ring ----
            wenc_sb = const.tile([128, N_GRP, 2, D_EMB], fp8)
            nc.scalar.dma_start(out=wenc_sb, in_=w_enc[:, :, :, :])
            cb16_sb = const.tile([128, 70], bf16)
            nc.scalar.dma_start(out=cb16_sb, in_=cb16[:, :])
            cf32_sb = const.tile([128, 12], f32)
            nc.scalar.dma_start(out=cf32_sb, in_=cf32[:, :])
            emit_x_mid(2, nc.scalar)
            emit_x_mid(1, nc.sync)
            emit_x_s3(nc.sync)

            watt_ap = cb16_sb[:, 0:64]
            ws2_ap = [cb16_sb[:, 64:66], cb16_sb[:, 66:68]]
            wcls_ap = cb16_sb[:, 68:70]
            benc_ap = cf32_sb[:, 0:1]
            batt2_ap = cf32_sb[:, 1:2]
            bcls_ap = cf32_sb[0:2, 2:10]

            den_all = const.tile([2, BAGS_PER_CORE], f32)
            sc_all = const.tile([2, BAGS_PER_CORE], f32)
            rden = const.tile([2, BAGS_PER_CORE], f32r)

            def emit_enc(s):
                pair = ps.tile([128, 2, INST_PER_BAG], f32, tag="e",
                               name=f"pse{s}")
                for g in range(N_GRP):
                    if s == N_SLABS - 1 and g == N_GRP - 1:
                        for h in range(2):
                            for m in range(2):
                                c0 = h * HALF
                                nc.tensor.matmul(
                                    pair[:, m, c0:c0 + HALF],
                                    wenc_sb[:, g, :, :],
                                    xg[s][g][:, :,
                                             m * INST_PER_BAG + c0:
                                             m * INST_PER_BAG + c0 + HALF],
                                    start=False, stop=True,
                                    perf_mode=DR, skip_group_check=True)
                    else:
                        for m in range(2):
                            nc.tensor.matmul(
                                pair[:, m, :],
                                wenc_sb[:, g, :, :],
                                xg[s][g][:, :, m * INST_PER_BAG:
                                         (m + 1) * INST_PER_BAG],
                                start=(g == 0), stop=(g == N_GRP - 1),
                                perf_mode=DR,
                                skip_group_check=(s == N_SLABS - 1))
                return pair

            def emit_head(s, pair, halve=False):
                """relu -> watt -> tanh (per column range)."""
                embT2 = work.tile([128, 2, INST_PER_BAG], bf16, tag="emb",
                                  name=f"emb{s}")
                ps_a = psa.tile([D_EMB, INST_PER_BAG], f32, tag="a",
                                name=f"psa{s}")
                aT2 = work.tile([128, INST_PER_BAG], bf16, tag="aT",
                                name=f"aT{s}")
                ranges = [slice(h * HALF, (h + 1) * HALF)
                          for h in range(2)] if halve \
                    else [slice(0, INST_PER_BAG)]
                for sl in ranges:
                    nc.scalar.activation(embT2[:, :, sl], pair[:, :, sl],
                                         AF.Relu, bias=benc_ap, scale=1.0)
                    nc.tensor.matmul(ps_a[0:64, sl], watt_ap,
                                     embT2[:, 0, sl], start=True, stop=True,
                                     tile_position=(0, 0))
                    nc.tensor.matmul(ps_a[64:128, sl], watt_ap,
                                     embT2[:, 1, sl], start=True, stop=True,
                                     tile_position=(0, 64))
                    nc.scalar.activation(aT2[:, sl], ps_a[:, sl], AF.Tanh,
                                         bias=batt2_ap, scale=1.0)
                return embT2, aT2

            def emit_bags(s, embT2, aT2, halve=False):
                """u -> ws2 -> exp -> score dot, per bag."""
                if halve:
                    denp = work.tile([2, 2, 2], f32, tag="denp",
                                     name=f"denp{s}")
                    scp = work.tile([2, 2, 2], f32, tag="scp",
                                    name=f"scp{s}")
                ranges = [slice(h * HALF, (h + 1) * HALF)
                          for h in range(2)] if halve \
                    else [slice(0, INST_PER_BAG)]
                us = {m: psu.tile([2, INST_PER_BAG], f32, tag="u",
                                  name=f"u{s}{m}") for m in range(2)}
                for hi, sl in enumerate(ranges):
                    for m in range(2):
                        nc.tensor.matmul(us[m][:, sl], wcls_ap,
                                         embT2[:, m, sl], start=True,
                                         stop=True, skip_group_check=True)
                    for m in range(2):
                        j = 2 * s + m
                        ps_l = psl.tile([2, INST_PER_BAG], f32, tag="l",
                                        name=f"psl{s}{m}")
                        e2 = work.tile([2, INST_PER_BAG], bf16,
                                       tag=f"e2{m}", name=f"e2{s}{m}")
                        nc.tensor.matmul(ps_l[:, sl], ws2_ap[m], aT2[:, sl],
                                         start=True, stop=True,
                                         skip_group_check=True)
                        acc = denp[:, m, hi:hi + 1] if halve \
                            else den_all[0:2, j:j + 1]
                        nc.scalar.activation(e2[:, sl], ps_l[:, sl], AF.Exp,
                                             scale=1.0, accum_out=acc)
                        sacc = scp[:, m, hi:hi + 1] if halve \
                            else sc_all[0:2, j:j + 1]
                        scr = work.tile([2, INST_PER_BAG], bf16, tag="scr",
                                        name=f"scr{s}{m}{hi}")
                        nc.vector.scalar_tensor_tensor(
                            out=scr[:, sl], in0=us[m][:, sl], scalar=1.0,
                            in1=e2[:, sl], op0=ALU.mult, op1=ALU.mult,
                            accum_out=sacc)
                if halve:
                    for m in range(2):
                        j = 2 * s + m
                        nc.vector.tensor_add(den_all[0:2, j:j + 1],
                                             denp[:, m, 0:1], denp[:, m, 1:2])
                        nc.vector.tensor_add(sc_all[0:2, j:j + 1],
                                             scp[:, m, 0:1], scp[:, m, 1:2])

            # order: enc s | head s | enc s+1 | bags s | head s+1 | ...
            prev = None
            for s in range(N_SLABS):
                pair = emit_enc(s)
                if prev is not None:
                    emit_bags(s - 1, *prev)
                    if s == N_SLABS - 1:
                        # reciprocal of the first 6 denominators off the
                        # critical drain chain
                        with nc.allow_low_precision(
                                reason="1/denom at f32r, ~1e-4 rel"):
                            nc.vector.reciprocal(rden[:, 0:6],
                                                 den_all[:, 0:6])
                prev = emit_head(s, pair, halve=(s == N_SLABS - 1))
            emit_bags(N_SLABS - 1, *prev, halve=True)

            # ---- score normalization + output ----
            with nc.allow_low_precision(reason="1/denom at f32r, ~1e-4 rel"):
                nc.vector.reciprocal(rden[:, 6:8], den_all[:, 6:8])
            s_n = const.tile([2, BAGS_PER_CORE], f32)
            nc.vector.tensor_mul(s_n, sc_all, rden)
            scores = const.tile([2, BAGS_PER_CORE], f32)
            nc.vector.tensor_add(scores, s_n, bcls_ap)
            nc.sync.dma_start(out=out[:, :], in_=scores)

    nc.compile()
    return nc


def _prep_shared(w_enc, b_enc, w_att, b_att, w_score, w_cls, b_cls):
    import ml_dtypes

    wenc_dr = np.ascontiguousarray(
        w_enc.reshape(N_GRP, 2, 128, D_EMB).transpose(2, 0, 1, 3)
    ).astype(ml_dtypes.float8_e4m3)

    cb16 = np.zeros((128, 70), dtype=np.float32)
    cb16[:, 0:64] = w_att
    cb16[0:64, 64] = w_score
    cb16[0:64, 65] = w_score
    cb16[64:128, 66] = w_score
    cb16[64:128, 67] = w_score
    cb16[:, 68:70] = w_cls
    cb16 = cb16.astype(ml_dtypes.bfloat16)

    cf32 = np.zeros((128, 12), dtype=np.float32)
    cf32[:, 0] = b_enc
    cf32[0:64, 1] = b_att
    cf32[64:128, 1] = b_att
    cf32[0:2, 2:10] = np.repeat(b_cls[:, None], BAGS_PER_CORE, axis=1)
    return {"w_enc": wenc_dr, "cb16": cb16, "cf32": cf32}


def make_in_maps(inputs):
    import ml_dtypes

    x = np.asarray(inputs["x"], dtype=np.float32)
    shared = _prep_shared(
        np.asarray(inputs["w_enc"], dtype=np.float32),
        np.asarray(inputs["b_enc"], dtype=np.float32),
        np.asarray(inputs["w_att"], dtype=np.float32),
        np.asarray(inputs["b_att"], dtype=np.float32),
        np.asarray(inputs["w_score"], dtype=np.float32),
        np.asarray(inputs["w_cls"], dtype=np.float32),
        np.asarray(inputs["b_cls"], dtype=np.float32),
    )
    in_maps = []
    for c in range(N_CORES):
        xs = x[c * INST_PER_CORE:(c + 1) * INST_PER_CORE]
        xt = np.ascontiguousarray(xs.T)
        np.clip(xt, -240.0, 240.0, out=xt)
        in_maps.append({"xt": xt.astype(ml_dtypes.float8_e4m3), **shared})
    return in_maps


def unpack_out(res):
    outs = []
    for c in range(N_CORES):
        o = np.asarray(res.results[c]["out"], dtype=np.float32)
        outs.append(o.T)                   # [2, 8] -> [8, 2]
    return np.ascontiguousarray(np.concatenate(outs, axis=0))


def _numpy_fallback(x, seg, w_enc, b_enc, w_att, b_att, w_score, b_score,
                    w_cls, b_cls):
    emb = np.maximum(x @ w_enc + b_enc, 0.0)
    a = np.tanh(emb @ w_att + b_att)
    logits = a @ w_score + b_score[0]
    out = np.zeros((N_BAGS, N_CLS), dtype=np.float32)
    for bag in range(N_BAGS):
        mask = seg == bag
        lg = logits[mask]
        e = np.exp(lg - lg.max())
        attn = e / e.sum()
        bag_emb = attn @ emb[mask]
        out[bag] = bag_emb @ w_cls + b_cls
    return out


def kernel(**inputs):
    from concourse.bass_utils import run_bass_kernel_spmd

    seg = np.asarray(inputs["seg"], dtype=np.int32)
    expected_seg = np.repeat(np.arange(N_BAGS, dtype=np.int32), INST_PER_BAG)
    if not np.array_equal(seg, expected_seg):
        return _numpy_fallback(
            np.asarray(inputs["x"], dtype=np.float32), seg,
            *[np.asarray(inputs[k], dtype=np.float32) for k in
              ("w_enc", "b_enc", "w_att", "b_att", "w_score", "b_score",
               "w_cls", "b_cls")])

    if "nc" not in _CACHE:
        _CACHE["nc"] = _build()
    nc = _CACHE["nc"]
    in_maps = make_in_maps(inputs)
    res = run_bass_kernel_spmd(nc, in_maps, core_ids=list(range(N_CORES)))
    return unpack_out(res)
